# revision 16
# baseline (speedup 1.0000x reference)
"""Tensor-parallel LlamaAttention (GQA + RoPE + causal) for 8 trn2 NeuronCores.

Sharding: column-parallel q/k/v by head (NH/8 q-heads, NKV/8 kv-heads per
core), attention computed locally per head, chunked AllGather of the
(transposed) attention output across cores overlapped with compute, then
column-parallel o_proj (each core computes a DQ-wide output-column slice);
host concatenates slices.

v3 design:
  - x pre-transposed on the host (xT[h, t]); x and weights packed so every
    DMA line is 4KB (4 h-blocks per SBUF tile) -> descriptor-rate no longer
    gates startup or per-chunk streaming.
  - softmax denominator L via two parallel DVE accumulation chains over the
    P tiles + ONE ones-matmul per (b, head, q-block).
  - causal trim on the diagonal 512-chunk (k-tile i only computes
    q-columns [i*128, 512); only the first 128-col block gets masked).
  - single pool scope, chunk-pipelined, THREE-way cost-weighted weave of
    [attention(c-1), projections(c), o_proj(c-2)] micro-units so the PE
    always has independent matmuls in flight during ACT/DVE latencies;
    gather(c-1) issued at iteration end, fully overlapped.

Layout (token dim = free dim everywhere except v/y):
  qT/kT[d, t] from projection matmuls (lhsT=W block, rhs=xT block) + rope
  v[t, d]     natural layout (lhsT=xT block, rhs=Wv block)
  S^T[k, q]   = matmul(lhsT=kT slice, rhs=qT slice)
  P^T         = exp(scale*S^T) via ACT (scores bounded, exp safe in f32)
  O^T[d, q]  += matmul(lhsT=v tile, rhs=P^T)   (PSUM accumulation over k)
  L[*, q]     = matmul(lhsT=ones, rhs=Pacc)
  attnT       = O^T * reciprocal(L) -> chunked AllGather -> o_proj lhsT
"""

import math
import sys
from contextlib import ExitStack

import numpy as np

sys.path.insert(0, "/opt/trn_rl_repo")

import ml_dtypes  # noqa: E402

from concourse import bacc, mybir, tile  # noqa: E402
from concourse.bass_utils import run_bass_kernel_spmd  # noqa: E402

F32 = mybir.dt.float32
BF16 = mybir.dt.bfloat16
NCORES = 8
P = 128  # partitions / head dim
QB = 512  # q-block / token chunk (PSUM free dim)
GP = 4  # h-blocks packed per DMA tile

_CACHE = {}


def _weave(streams):
    """Merge lists of (cost, fn) preserving intra-list order, interleaved by
    cumulative-cost fraction, and call the fns."""
    streams = [s for s in streams if s]
    totals = [max(1, sum(c for c, _ in s)) for s in streams]
    done = [0.0] * len(streams)
    idx = [0] * len(streams)
    while True:
        best, bf = -1, None
        for si, s in enumerate(streams):
            if idx[si] < len(s):
                f = done[si] / totals[si]
                if bf is None or f < bf:
                    best, bf = si, f
        if best < 0:
            break
        c, fn = streams[best][idx[best]]
        idx[best] += 1
        done[best] += c
        fn()


def build_program(B, S, H, NH, NKV):
    """Build the per-core Bass program. All cores run the same program on
    different weight slices."""
    nc = bacc.Bacc("TRN2", num_devices=NCORES)

    BT = B * S  # total tokens
    NHC = NH // NCORES  # q heads per core
    NKC = NKV // NCORES  # kv heads per core
    assert NKC == 1
    DQ = NHC * P  # per-core q width
    HB = H // P  # h blocks (contraction tiles)
    HG = HB // GP  # packed h groups
    assert HG * GP == HB
    TB = BT // QB  # token chunks
    QBB = S // QB  # q chunks per batch
    KPC = QB // P  # k-tiles (128) per chunk (4)

    xT_d = nc.declare_dram_parameter("xT_d", [HG * P, TB * GP * QB], BF16, isOutput=False)
    wq_c = nc.declare_dram_parameter("wq_c", [HG * P, GP * DQ], BF16, isOutput=False)
    wk_c = nc.declare_dram_parameter("wk_c", [HG * P, GP * P], BF16, isOutput=False)
    wv_c = nc.declare_dram_parameter("wv_c", [HG * P, GP * P], BF16, isOutput=False)
    wo_c = nc.declare_dram_parameter("wo_c", [HG * P, GP * DQ], BF16, isOutput=False)
    cos_t = nc.declare_dram_parameter("cos_t", [P, BT], F32, isOutput=False)
    sinx_t = nc.declare_dram_parameter("sinx_t", [P, BT], F32, isOutput=False)
    consts_t = nc.declare_dram_parameter("consts_t", [2, P, P], BF16, isOutput=False)
    y_c = nc.declare_dram_parameter("y_c", [BT, DQ], F32, isOutput=True)

    scale = 1.0 / math.sqrt(P)

    with tile.TileContext(nc) as tc:
        with ExitStack() as stack:
            pool = lambda *a, **kw: stack.enter_context(tc.tile_pool(*a, **kw))
            dram = pool(name="dram", bufs=1, space="DRAM")
            constp = pool(name="const", bufs=1)
            w_p = pool(name="wpool", bufs=1)
            kv_p = pool(name="kvq", bufs=1)
            qt_p = pool(name="qt", bufs=2)
            xt_p = pool(name="xt", bufs=10)
            tab_p = pool(name="tabs", bufs=2)
            rt_p = pool(name="ropetmp", bufs=1)
            p_p = pool(name="pP", bufs=6)
            pa_p = pool(name="pacc", bufs=2)
            pa2_p = pool(name="pacc2", bufs=2)
            pab_p = pool(name="paccb", bufs=2)
            ri_p = pool(name="rinvp", bufs=2)
            ao_p = pool(name="aout", bufs=2)
            as_p = pool(name="astr", bufs=8)
            y_p = pool(name="yout", bufs=2)
            psp_p = pool(name="psP", bufs=2, space="PSUM")
            pss_p = pool(name="psS", bufs=2, space="PSUM")
            pso_p = pool(name="psO", bufs=2, space="PSUM")
            psy_p = pool(name="psY", bufs=2, space="PSUM")

            attn_loc = [
                dram.tile([DQ, QB], BF16, tag=f"aloc{c}", name=f"aloc{c}")
                for c in range(TB)
            ]
            attn_full = [
                dram.tile([NCORES * DQ, QB], BF16, tag=f"afull{c}", name=f"afull{c}")
                for c in range(TB)
            ]

            tri_sb = constp.tile([P, P], BF16, tag="tri")
            nc.sync.dma_start(out=tri_sb, in_=consts_t[0])
            ones_sb = constp.tile([P, P], BF16, tag="ones")
            nc.sync.dma_start(out=ones_sb, in_=consts_t[1])

            # weights resident in SBUF, packed GP h-blocks per tile (4KB lines)
            wq_sb = [w_p.tile([P, GP * DQ], BF16, tag=f"wq{g}", name=f"wq{g}") for g in range(HG)]
            wk_sb = [w_p.tile([P, GP * P], BF16, tag=f"wk{g}", name=f"wk{g}") for g in range(HG)]
            wv_sb = [w_p.tile([P, GP * P], BF16, tag=f"wv{g}", name=f"wv{g}") for g in range(HG)]
            wo_sb = [w_p.tile([P, GP * DQ], BF16, tag=f"wo{g}", name=f"wo{g}") for g in range(HG)]
            for g in range(HG):
                nc.sync.dma_start(out=wq_sb[g], in_=wq_c[g * P : (g + 1) * P, :])
                nc.sync.dma_start(out=wk_sb[g], in_=wk_c[g * P : (g + 1) * P, :])
                nc.sync.dma_start(out=wv_sb[g], in_=wv_c[g * P : (g + 1) * P, :])
                nc.sync.dma_start(out=wo_sb[g], in_=wo_c[g * P : (g + 1) * P, :])

            # persistent per-chunk activations (bf16)
            kT = [kv_p.tile([P, QB], BF16, tag=f"kT{c}", name=f"kT{c}") for c in range(TB)]
            vsb = [kv_p.tile([P, QB], BF16, tag=f"v{c}", name=f"v{c}") for c in range(TB)]
            qt_tiles = {}
            xt_tiles = {}

            def rope(dst, ps, cos_sb, sinx_sb):
                """dst = ps*cos + shift64(ps)*sinx (all [128,QB])"""
                t1 = rt_p.tile([P, QB], F32, tag="ropet1")
                t2 = rt_p.tile([P, QB], F32, tag="ropet2")
                nc.vector.tensor_tensor(t1, ps, cos_sb, mybir.AluOpType.mult)
                hf = P // 2
                nc.vector.tensor_tensor(t2[0:hf], ps[hf:P], sinx_sb[0:hf], mybir.AluOpType.mult)
                nc.vector.tensor_tensor(t2[hf:P], ps[0:hf], sinx_sb[hf:P], mybir.AluOpType.mult)
                nc.vector.tensor_tensor(dst, t1, t2, mybir.AluOpType.add)

            def proj_micros(c):
                """(cost, fn) micro-units for projections of chunk c."""
                t0 = c * GP * QB
                cos_sb = tab_p.tile([P, QB], F32, tag="cos")
                sinx_sb = tab_p.tile([P, QB], F32, tag="sinx")
                nc.sync.dma_start(out=cos_sb, in_=cos_t[:, c * QB : (c + 1) * QB])
                nc.sync.dma_start(out=sinx_sb, in_=sinx_t[:, c * QB : (c + 1) * QB])
                xts = []
                for g in range(HG):
                    xi = xt_p.tile([P, GP * QB], BF16, tag="xt")
                    nc.sync.dma_start(
                        out=xi, in_=xT_d[g * P : (g + 1) * P, t0 : t0 + GP * QB]
                    )
                    xts.append(xi)
                xt_tiles[c] = xts

                def xs(hb):  # rhs [P, QB] for h-block hb
                    g, sub = divmod(hb, GP)
                    return xts[g][:, sub * QB : (sub + 1) * QB]

                qts = []
                for h in range(NHC):
                    qts.append(qt_p.tile([P, QB], BF16, tag=f"qT{h}", name=f"qT{h}"))
                qt_tiles[c] = qts

                micros = []
                # q projections, 4-MM micro-units
                state = {}

                def q_mm(h, hb0):
                    if hb0 == 0:
                        state[h] = psp_p.tile([P, QB], F32, tag="pps", name=f"qps{h}")
                    q_ps = state[h]
                    for hb in range(hb0, hb0 + GP):
                        g, sub = divmod(hb, GP)
                        nc.tensor.matmul(
                            q_ps,
                            wq_sb[g][:, sub * DQ + h * P : sub * DQ + (h + 1) * P],
                            xs(hb),
                            start=(hb == 0),
                            stop=(hb == HB - 1),
                        )
                    if hb0 + GP == HB:
                        rope(qts[h], q_ps, cos_sb, sinx_sb)

                for h in range(NHC):
                    for hb0 in range(0, HB, GP):
                        micros.append((GP * QB, lambda h=h, hb0=hb0: q_mm(h, hb0)))

                def k_mm(hb0):
                    if hb0 == 0:
                        state["k"] = psp_p.tile([P, QB], F32, tag="pps", name="kps")
                    k_ps = state["k"]
                    for hb in range(hb0, hb0 + GP):
                        g, sub = divmod(hb, GP)
                        nc.tensor.matmul(
                            k_ps,
                            wk_sb[g][:, sub * P : (sub + 1) * P],
                            xs(hb),
                            start=(hb == 0),
                            stop=(hb == HB - 1),
                        )
                    if hb0 + GP == HB:
                        rope(kT[c], k_ps, cos_sb, sinx_sb)

                for hb0 in range(0, HB, GP):
                    micros.append((GP * QB, lambda hb0=hb0: k_mm(hb0)))

                def v_mm(i, hb0):
                    if i == 0 and hb0 == 0:
                        state["v"] = psp_p.tile([P, QB], F32, tag="pps", name="vps")
                    v_ps = state["v"]
                    for hb in range(hb0, hb0 + GP):
                        g, sub = divmod(hb, GP)
                        nc.tensor.matmul(
                            v_ps[:, i * P : (i + 1) * P],
                            xts[g][:, sub * QB + i * P : sub * QB + (i + 1) * P],
                            wv_sb[g][:, sub * P : (sub + 1) * P],
                            start=(hb == 0),
                            stop=(hb == HB - 1),
                        )
                    if i == KPC - 1 and hb0 + GP == HB:
                        nc.vector.tensor_copy(vsb[c], v_ps)

                for i in range(KPC):
                    for hb0 in range(0, HB, GP):
                        micros.append((GP * P, lambda i=i, hb0=hb0: v_mm(i, hb0)))
                return micros

            def attn_micros(ca):
                """(cost, fn) micro-units for attention of chunk ca."""
                b, qb = divmod(ca, QBB)
                micros = []

                def kb_unit(st, h, j, kc, i, fs, nlast, h2):
                    if j == 0:
                        st["o"] = pso_p.tile([P, QB], F32, tag="ops", name="ops")
                        st["a0"] = pa_p.tile([P, QB], F32, tag="pacc", name="pacc")
                        st["a1"] = pa2_p.tile([P, QB], F32, tag="pacc2", name="pacc2")
                    kch = b * QBB + kc
                    s_ps = pss_p.tile([P, QB], F32, tag="sps")
                    nc.tensor.matmul(
                        s_ps[:, fs:],
                        kT[kch][:, i * P : (i + 1) * P],
                        qt_tiles[ca][h][:, fs:],
                        start=True,
                        stop=True,
                    )
                    p_sb = p_p.tile([P, QB], BF16, tag="P")
                    nc.scalar.activation(
                        p_sb[:, fs:],
                        s_ps[:, fs:],
                        mybir.ActivationFunctionType.Exp,
                        scale=scale,
                    )
                    if kc == qb:  # diagonal chunk: mask first 128-col block
                        nc.vector.tensor_tensor(
                            p_sb[:, fs : fs + P],
                            p_sb[:, fs : fs + P],
                            tri_sb,
                            mybir.AluOpType.mult,
                        )
                    acc = st["a0"] if j < h2 else st["a1"]
                    if j == 0 or j == h2:
                        nc.vector.tensor_copy(acc[:, fs:], p_sb[:, fs:])
                    else:
                        nc.vector.tensor_tensor(
                            acc[:, fs:], acc[:, fs:], p_sb[:, fs:], mybir.AluOpType.add
                        )
                    nc.tensor.matmul(
                        st["o"][:, fs:],
                        vsb[kch][:, i * P : (i + 1) * P],
                        p_sb[:, fs:],
                        start=(j == 0),
                        stop=(j == nlast),
                    )

                def head_tail(st, h, fs_h2):
                    nc.vector.tensor_tensor(
                        st["a0"][:, fs_h2:],
                        st["a0"][:, fs_h2:],
                        st["a1"][:, fs_h2:],
                        mybir.AluOpType.add,
                    )
                    pacc_bf = pab_p.tile([P, QB], BF16, tag="paccb")
                    nc.vector.tensor_copy(pacc_bf, st["a0"])
                    l_ps = pss_p.tile([P, QB], F32, tag="sps")
                    nc.tensor.matmul(l_ps, ones_sb, pacc_bf, start=True, stop=True)
                    rinv = ri_p.tile([P, QB], F32, tag="rinv")
                    nc.vector.reciprocal(rinv, l_ps)
                    attn_sb = ao_p.tile([P, QB], BF16, tag="attn")
                    nc.vector.tensor_tensor(attn_sb, st["o"], rinv, mybir.AluOpType.mult)
                    nc.sync.dma_start(
                        out=attn_loc[ca][h * P : (h + 1) * P, :], in_=attn_sb
                    )

                for h in range(NHC):
                    tiles = [(kc, i, 0) for kc in range(qb) for i in range(KPC)]
                    tiles += [(qb, i, i * P) for i in range(KPC)]
                    nlast = len(tiles) - 1
                    h2 = len(tiles) // 2
                    fs_h2 = tiles[h2][2]
                    st = {}
                    for j, (kc, i, fs) in enumerate(tiles):
                        micros.append(
                            (
                                2 * (QB - fs),
                                lambda st=st, h=h, j=j, kc=kc, i=i, fs=fs, nlast=nlast, h2=h2: kb_unit(
                                    st, h, j, kc, i, fs, nlast, h2
                                ),
                            )
                        )
                    micros.append(
                        (QB, lambda st=st, h=h, fs_h2=fs_h2: head_tail(st, h, fs_h2))
                    )
                return micros

            def oproj_micros(co):
                """(cost, fn) micro-units for o_proj of chunk co.

                ha-outer over tsub pairs: each attn tile is consumed by two
                matmuls right after its DMA, so the stream pool stays tiny."""
                t0 = co * QB
                st = {}
                micros = []

                def o_mm(pair, ha0):
                    ta, tb = 2 * pair, 2 * pair + 1
                    if ha0 == 0:
                        st["ya"] = psy_p.tile([P, DQ], F32, tag="yps", name="yps")
                        st["yb"] = psy_p.tile([P, DQ], F32, tag="yps", name="yps2")
                    for ha in range(ha0, ha0 + GP):
                        a = as_p.tile([P, QB], BF16, tag="astr")
                        nc.sync.dma_start(
                            out=a, in_=attn_full[co][ha * P : (ha + 1) * P, :]
                        )
                        g, sub = divmod(ha, GP)
                        nc.tensor.matmul(
                            st["ya"],
                            a[:, ta * P : (ta + 1) * P],
                            wo_sb[g][:, sub * DQ : (sub + 1) * DQ],
                            start=(ha == 0),
                            stop=(ha == HB - 1),
                        )
                        nc.tensor.matmul(
                            st["yb"],
                            a[:, tb * P : (tb + 1) * P],
                            wo_sb[g][:, sub * DQ : (sub + 1) * DQ],
                            start=(ha == 0),
                            stop=(ha == HB - 1),
                        )
                    if ha0 + GP == HB:
                        for tsub, key in ((ta, "ya"), (tb, "yb")):
                            y_sb = y_p.tile([P, DQ], F32, tag="ysb", name=f"ysb{tsub}")
                            if tsub % 2 == 0:
                                nc.scalar.copy(y_sb, st[key])
                            else:
                                nc.vector.tensor_copy(y_sb, st[key])
                            nc.sync.dma_start(
                                out=y_c[t0 + tsub * P : t0 + (tsub + 1) * P, :],
                                in_=y_sb,
                            )

                for pair in range(KPC // 2):
                    for ha0 in range(0, HB, GP):
                        micros.append((2 * GP * DQ, lambda p=pair, a=ha0: o_mm(p, a)))
                return micros

            for it in range(TB + 2):
                c, ca, co = it, it - 1, it - 2
                streams = []
                if 0 <= ca < TB:
                    streams.append(attn_micros(ca))
                if c < TB:
                    streams.append(proj_micros(c))
                if 0 <= co < TB:
                    streams.append(oproj_micros(co))
                _weave(streams)
                if 0 <= ca < TB:
                    nc.gpsimd.collective_compute(
                        "AllGather",
                        mybir.AluOpType.bypass,
                        replica_groups=[list(range(NCORES))],
                        ins=[attn_loc[ca][:, :]],
                        outs=[attn_full[ca][:, :]],
                    )
                if ca - 1 >= 0:
                    del qt_tiles[ca - 1]
                    del xt_tiles[ca - 1]
    nc.finalize()
    return nc


def _prep_inputs(hidden_states, wq, wk, wv, wo, position_ids, B, S, H, NH, NKV):
    """Host-side: bf16 casts, x pre-transpose, packed layouts, rope tables."""
    BT = B * S
    NHC = NH // NCORES
    DQ = NHC * P
    HB = H // P
    HG = HB // GP
    TB = BT // QB

    bf = ml_dtypes.bfloat16

    def pack_w(w):
        # [H, D] -> [HG*P, GP*D]: row g*P+p, col sub*D+d = w[(g*GP+sub)*P+p, d]
        D = w.shape[1]
        return np.ascontiguousarray(
            w.reshape(HG, GP, P, D).transpose(0, 2, 1, 3).reshape(HG * P, GP * D)
        )

    xT = np.asarray(hidden_states).reshape(BT, H).T.astype(bf)  # [H, BT]
    # [H, BT] -> [HG*P, TB*GP*QB]: row g*P+p, col c*GP*QB + sub*QB + t
    #   = xT[(g*GP+sub)*P + p, c*QB + t]
    xTp = np.ascontiguousarray(
        xT.reshape(HG, GP, P, TB, QB).transpose(0, 2, 3, 1, 4).reshape(HG * P, TB * GP * QB)
    )
    wq_b, wk_b, wv_b, wo_b = (np.asarray(w).astype(bf) for w in (wq, wk, wv, wo))

    # rope tables in transposed layout [128 d, BT t]
    half = P // 2
    inv_freq = 1.0 / (10000.0 ** (np.arange(half, dtype=np.float64) / half))
    pos = np.asarray(position_ids).astype(np.float64).reshape(BT)  # [b*S+s]
    ang = pos[None, :] * inv_freq[:, None]  # [64, BT]
    cos_t = np.concatenate([np.cos(ang), np.cos(ang)], 0).astype(np.float32)
    sinx_t = np.concatenate([-np.sin(ang), np.sin(ang)], 0).astype(np.float32)

    # tri[k, q] = 1 if k <= q (diagonal 128x128 causal block)
    kk = np.arange(P)[:, None]
    qq = np.arange(P)[None, :]
    tri = (kk <= qq).astype(bf)
    consts_t = np.stack([tri, np.ones((P, P), dtype=bf)]).astype(bf)

    in_maps = []
    for c in range(NCORES):
        in_maps.append(
            {
                "xT_d": xTp,
                "wq_c": pack_w(wq_b[:, c * DQ : (c + 1) * DQ]),
                "wk_c": pack_w(wk_b[:, c * P : (c + 1) * P]),
                "wv_c": pack_w(wv_b[:, c * P : (c + 1) * P]),
                "wo_c": pack_w(wo_b[:, c * DQ : (c + 1) * DQ]),
                "cos_t": cos_t,
                "sinx_t": sinx_t,
                "consts_t": consts_t,
            }
        )
    return in_maps


def run(hidden_states, wq, wk, wv, wo, position_ids, B, S, H, NH, NKV, trace=False):
    key = (B, S, H, NH, NKV)
    if key not in _CACHE:
        _CACHE[key] = build_program(B, S, H, NH, NKV)
    nc = _CACHE[key]
    in_maps = _prep_inputs(
        hidden_states, wq, wk, wv, wo, position_ids, B, S, H, NH, NKV
    )
    res = run_bass_kernel_spmd(nc, in_maps, core_ids=list(range(NCORES)), trace=trace)
    y = np.concatenate([res.results[c]["y_c"] for c in range(NCORES)], axis=1)
    out = y.reshape(B, S, NH * P).astype(np.float32)
    return (out, res) if trace else (out, None)


def kernel(hidden_states, wq, wk, wv, wo, position_ids):
    out, _ = run(
        hidden_states, wq, wk, wv, wo, position_ids, 2, 2048, 4096, 32, 8
    )
    return out


# revision 20
# speedup vs baseline: 1.0883x; 1.0883x over previous
"""Tensor-parallel LlamaAttention (GQA + RoPE + causal) for 8 trn2 NeuronCores.

Sharding: column-parallel q/k/v by head (NH/8 q-heads, NKV/8 kv-heads per
core), attention computed locally per head, chunked AllGather of the
(transposed) attention output across cores overlapped with compute, then
column-parallel o_proj (each core computes a DQ-wide output-column slice);
host concatenates slices.

v3 design:
  - x pre-transposed on the host (xT[h, t]); x and weights packed so every
    DMA line is 4KB (4 h-blocks per SBUF tile) -> descriptor-rate no longer
    gates startup or per-chunk streaming.
  - softmax denominator L via two parallel DVE accumulation chains over the
    P tiles + ONE ones-matmul per (b, head, q-block).
  - causal trim on the diagonal 512-chunk (k-tile i only computes
    q-columns [i*128, 512); only the first 128-col block gets masked).
  - single pool scope, chunk-pipelined, THREE-way cost-weighted weave of
    [attention(c-1), projections(c), o_proj(c-2)] micro-units so the PE
    always has independent matmuls in flight during ACT/DVE latencies;
    gather(c-1) issued at iteration end, fully overlapped.

Layout (token dim = free dim everywhere except v/y):
  qT/kT[d, t] from projection matmuls (lhsT=W block, rhs=xT block) + rope
  v[t, d]     natural layout (lhsT=xT block, rhs=Wv block)
  S^T[k, q]   = matmul(lhsT=kT slice, rhs=qT slice)
  P^T         = exp(scale*S^T) via ACT (scores bounded, exp safe in f32)
  O^T[d, q]  += matmul(lhsT=v tile, rhs=P^T)   (PSUM accumulation over k)
  L[*, q]     = matmul(lhsT=ones, rhs=Pacc)
  attnT       = O^T * reciprocal(L) -> chunked AllGather -> o_proj lhsT
"""

import math
import sys
from contextlib import ExitStack

import numpy as np

sys.path.insert(0, "/opt/trn_rl_repo")

import ml_dtypes  # noqa: E402

from concourse import bacc, mybir, tile  # noqa: E402
from concourse.bass_utils import run_bass_kernel_spmd  # noqa: E402

F32 = mybir.dt.float32
BF16 = mybir.dt.bfloat16
NCORES = 8
P = 128  # partitions / head dim
QB = 512  # q-block / token chunk (PSUM free dim)
GP = 4  # h-blocks packed per DMA tile

_CACHE = {}


def _weave(streams):
    """Merge [(units, weight, offset), ...] preserving intra-list order,
    interleaved by cumulative-cost fraction, and call the fns.

    weight > 1 makes a stream finish earlier (its fraction advances slower
    per unit); offset > 0 holds a stream back until the others reach that
    fraction of their own progress."""
    streams = [s for s in streams if s[0]]
    denoms = [max(1, sum(c for c, _ in s[0])) * wt for s, wt in ((s, s[1]) for s in streams)]
    done = [s[2] * d for s, d in zip(streams, denoms)]
    idx = [0] * len(streams)
    while True:
        best, bf = -1, None
        for si, (s, _, _) in enumerate(streams):
            if idx[si] < len(s):
                f = done[si] / denoms[si]
                if bf is None or f < bf:
                    best, bf = si, f
        if best < 0:
            break
        c, fn = streams[best][0][idx[best]]
        idx[best] += 1
        done[best] += c
        fn()


def build_program(B, S, H, NH, NKV):
    """Build the per-core Bass program. All cores run the same program on
    different weight slices."""
    nc = bacc.Bacc("TRN2", num_devices=NCORES)

    BT = B * S  # total tokens
    NHC = NH // NCORES  # q heads per core
    NKC = NKV // NCORES  # kv heads per core
    assert NKC == 1
    DQ = NHC * P  # per-core q width
    HB = H // P  # h blocks (contraction tiles)
    HG = HB // GP  # packed h groups
    assert HG * GP == HB
    TB = BT // QB  # token chunks
    QBB = S // QB  # q chunks per batch
    KPC = QB // P  # k-tiles (128) per chunk (4)

    xT_d = nc.declare_dram_parameter("xT_d", [HG * P, TB * GP * QB], BF16, isOutput=False)
    wq_c = nc.declare_dram_parameter("wq_c", [HG * P, GP * DQ], BF16, isOutput=False)
    wk_c = nc.declare_dram_parameter("wk_c", [HG * P, GP * P], BF16, isOutput=False)
    wv_c = nc.declare_dram_parameter("wv_c", [HG * P, GP * P], BF16, isOutput=False)
    wo_c = nc.declare_dram_parameter("wo_c", [HG * P, GP * DQ], BF16, isOutput=False)
    cos_t = nc.declare_dram_parameter("cos_t", [P, BT], F32, isOutput=False)
    sinx_t = nc.declare_dram_parameter("sinx_t", [P, BT], F32, isOutput=False)
    consts_t = nc.declare_dram_parameter("consts_t", [2, P, P], BF16, isOutput=False)
    y_c = nc.declare_dram_parameter("y_c", [BT, DQ], F32, isOutput=True)

    scale = 1.0 / math.sqrt(P)

    with tile.TileContext(nc) as tc:
        with ExitStack() as stack:
            pool = lambda *a, **kw: stack.enter_context(tc.tile_pool(*a, **kw))
            dram = pool(name="dram", bufs=1, space="DRAM")
            constp = pool(name="const", bufs=1)
            w_p = pool(name="wpool", bufs=1)
            kv_p = pool(name="kvq", bufs=1)
            qt_p = pool(name="qt", bufs=2)
            xt_p = pool(name="xt", bufs=10)
            tab_p = pool(name="tabs", bufs=2)
            rt_p = pool(name="ropetmp", bufs=1)
            p_p = pool(name="pP", bufs=6)
            pa_p = pool(name="pacc", bufs=2)
            pa2_p = pool(name="pacc2", bufs=2)
            pab_p = pool(name="paccb", bufs=2)
            ri_p = pool(name="rinvp", bufs=2)
            ao_p = pool(name="aout", bufs=2)
            as_p = pool(name="astr", bufs=12)
            y_p = pool(name="yout", bufs=2)
            psp_p = pool(name="psP", bufs=2, space="PSUM")
            pss_p = pool(name="psS", bufs=2, space="PSUM")
            pso_p = pool(name="psO", bufs=2, space="PSUM")
            psy_p = pool(name="psY", bufs=2, space="PSUM")

            attn_loc = [
                dram.tile([DQ, QB], BF16, tag=f"aloc{c}", name=f"aloc{c}")
                for c in range(TB)
            ]
            attn_full = [
                dram.tile([NCORES * DQ, QB], BF16, tag=f"afull{c}", name=f"afull{c}")
                for c in range(TB)
            ]

            tri_sb = constp.tile([P, P], BF16, tag="tri")
            nc.sync.dma_start(out=tri_sb, in_=consts_t[0])
            ones_sb = constp.tile([P, P], BF16, tag="ones")
            nc.sync.dma_start(out=ones_sb, in_=consts_t[1])

            # weights resident in SBUF, packed GP h-blocks per tile (4KB lines)
            wq_sb = [w_p.tile([P, GP * DQ], BF16, tag=f"wq{g}", name=f"wq{g}") for g in range(HG)]
            wk_sb = [w_p.tile([P, GP * P], BF16, tag=f"wk{g}", name=f"wk{g}") for g in range(HG)]
            wv_sb = [w_p.tile([P, GP * P], BF16, tag=f"wv{g}", name=f"wv{g}") for g in range(HG)]
            wo_sb = [w_p.tile([P, GP * DQ], BF16, tag=f"wo{g}", name=f"wo{g}") for g in range(HG)]
            for g in range(HG):
                nc.sync.dma_start(out=wq_sb[g], in_=wq_c[g * P : (g + 1) * P, :])

            # persistent per-chunk activations (bf16)
            kT = [kv_p.tile([P, QB], BF16, tag=f"kT{c}", name=f"kT{c}") for c in range(TB)]
            vsb = [kv_p.tile([P, QB], BF16, tag=f"v{c}", name=f"v{c}") for c in range(TB)]
            qt_tiles = {}
            xt_tiles = {}

            def rope(dst, ps, cos_sb, sinx_sb):
                """dst = ps*cos + shift64(ps)*sinx (all [128,QB])"""
                t1 = rt_p.tile([P, QB], F32, tag="ropet1")
                t2 = rt_p.tile([P, QB], F32, tag="ropet2")
                nc.vector.tensor_tensor(t1, ps, cos_sb, mybir.AluOpType.mult)
                hf = P // 2
                nc.vector.tensor_tensor(t2[0:hf], ps[hf:P], sinx_sb[0:hf], mybir.AluOpType.mult)
                nc.vector.tensor_tensor(t2[hf:P], ps[0:hf], sinx_sb[hf:P], mybir.AluOpType.mult)
                nc.vector.tensor_tensor(dst, t1, t2, mybir.AluOpType.add)

            def proj_micros(c):
                """(cost, fn) micro-units for projections of chunk c."""
                t0 = c * GP * QB
                cos_sb = tab_p.tile([P, QB], F32, tag="cos")
                sinx_sb = tab_p.tile([P, QB], F32, tag="sinx")
                nc.sync.dma_start(out=cos_sb, in_=cos_t[:, c * QB : (c + 1) * QB])
                nc.sync.dma_start(out=sinx_sb, in_=sinx_t[:, c * QB : (c + 1) * QB])
                xts = []
                for g in range(HG):
                    xi = xt_p.tile([P, GP * QB], BF16, tag="xt")
                    nc.sync.dma_start(
                        out=xi, in_=xT_d[g * P : (g + 1) * P, t0 : t0 + GP * QB]
                    )
                    xts.append(xi)
                xt_tiles[c] = xts

                def xs(hb):  # rhs [P, QB] for h-block hb
                    g, sub = divmod(hb, GP)
                    return xts[g][:, sub * QB : (sub + 1) * QB]

                qts = []
                for h in range(NHC):
                    qts.append(qt_p.tile([P, QB], BF16, tag=f"qT{h}", name=f"qT{h}"))
                qt_tiles[c] = qts

                micros = []
                # q projections, 4-MM micro-units
                state = {}

                def q_mm(h, hb0):
                    if hb0 == 0:
                        state[h] = psp_p.tile([P, QB], F32, tag="pps", name=f"qps{h}")
                    q_ps = state[h]
                    for hb in range(hb0, hb0 + GP):
                        g, sub = divmod(hb, GP)
                        nc.tensor.matmul(
                            q_ps,
                            wq_sb[g][:, sub * DQ + h * P : sub * DQ + (h + 1) * P],
                            xs(hb),
                            start=(hb == 0),
                            stop=(hb == HB - 1),
                        )
                    if hb0 + GP == HB:
                        rope(qts[h], q_ps, cos_sb, sinx_sb)

                for h in range(NHC):
                    for hb0 in range(0, HB, GP):
                        micros.append((GP * QB, lambda h=h, hb0=hb0: q_mm(h, hb0)))

                def k_mm(hb0):
                    if hb0 == 0:
                        state["k"] = psp_p.tile([P, QB], F32, tag="pps", name="kps")
                    k_ps = state["k"]
                    for hb in range(hb0, hb0 + GP):
                        g, sub = divmod(hb, GP)
                        nc.tensor.matmul(
                            k_ps,
                            wk_sb[g][:, sub * P : (sub + 1) * P],
                            xs(hb),
                            start=(hb == 0),
                            stop=(hb == HB - 1),
                        )
                    if hb0 + GP == HB:
                        rope(kT[c], k_ps, cos_sb, sinx_sb)

                for hb0 in range(0, HB, GP):
                    micros.append((GP * QB, lambda hb0=hb0: k_mm(hb0)))

                def v_mm(i, hb0):
                    if i == 0 and hb0 == 0:
                        state["v"] = psp_p.tile([P, QB], F32, tag="pps", name="vps")
                    v_ps = state["v"]
                    for hb in range(hb0, hb0 + GP):
                        g, sub = divmod(hb, GP)
                        nc.tensor.matmul(
                            v_ps[:, i * P : (i + 1) * P],
                            xts[g][:, sub * QB + i * P : sub * QB + (i + 1) * P],
                            wv_sb[g][:, sub * P : (sub + 1) * P],
                            start=(hb == 0),
                            stop=(hb == HB - 1),
                        )
                    if i == KPC - 1 and hb0 + GP == HB:
                        nc.vector.tensor_copy(vsb[c], v_ps)

                for i in range(KPC):
                    for hb0 in range(0, HB, GP):
                        micros.append((GP * P, lambda i=i, hb0=hb0: v_mm(i, hb0)))
                return micros

            def attn_micros(ca):
                """(cost, fn) micro-units for attention of chunk ca."""
                b, qb = divmod(ca, QBB)
                micros = []

                def kb_unit(st, h, j, kc, i, fs, nlast, h2):
                    if j == 0:
                        st["o"] = pso_p.tile([P, QB], F32, tag="ops", name="ops")
                        st["a0"] = pa_p.tile([P, QB], F32, tag="pacc", name="pacc")
                        st["a1"] = pa2_p.tile([P, QB], F32, tag="pacc2", name="pacc2")
                    kch = b * QBB + kc
                    s_ps = pss_p.tile([P, QB], F32, tag="sps")
                    nc.tensor.matmul(
                        s_ps[:, fs:],
                        kT[kch][:, i * P : (i + 1) * P],
                        qt_tiles[ca][h][:, fs:],
                        start=True,
                        stop=True,
                    )
                    p_sb = p_p.tile([P, QB], BF16, tag="P")
                    nc.scalar.activation(
                        p_sb[:, fs:],
                        s_ps[:, fs:],
                        mybir.ActivationFunctionType.Exp,
                        scale=scale,
                    )
                    if kc == qb:  # diagonal chunk: mask first 128-col block
                        nc.vector.tensor_tensor(
                            p_sb[:, fs : fs + P],
                            p_sb[:, fs : fs + P],
                            tri_sb,
                            mybir.AluOpType.mult,
                        )
                    acc = st["a0"] if j < h2 else st["a1"]
                    if j == 0 or j == h2:
                        nc.vector.tensor_copy(acc[:, fs:], p_sb[:, fs:])
                    else:
                        nc.vector.tensor_tensor(
                            acc[:, fs:], acc[:, fs:], p_sb[:, fs:], mybir.AluOpType.add
                        )
                    nc.tensor.matmul(
                        st["o"][:, fs:],
                        vsb[kch][:, i * P : (i + 1) * P],
                        p_sb[:, fs:],
                        start=(j == 0),
                        stop=(j == nlast),
                    )

                def head_tail(st, h, fs_h2):
                    nc.vector.tensor_tensor(
                        st["a0"][:, fs_h2:],
                        st["a0"][:, fs_h2:],
                        st["a1"][:, fs_h2:],
                        mybir.AluOpType.add,
                    )
                    pacc_bf = pab_p.tile([P, QB], BF16, tag="paccb")
                    nc.vector.tensor_copy(pacc_bf, st["a0"])
                    l_ps = pss_p.tile([P, QB], F32, tag="sps")
                    nc.tensor.matmul(l_ps, ones_sb, pacc_bf, start=True, stop=True)
                    rinv = ri_p.tile([P, QB], F32, tag="rinv")
                    nc.vector.reciprocal(rinv, l_ps)
                    attn_sb = ao_p.tile([P, QB], BF16, tag="attn")
                    nc.vector.tensor_tensor(attn_sb, st["o"], rinv, mybir.AluOpType.mult)
                    nc.sync.dma_start(
                        out=attn_loc[ca][h * P : (h + 1) * P, :], in_=attn_sb
                    )

                for h in range(NHC):
                    tiles = [(kc, i, 0) for kc in range(qb) for i in range(KPC)]
                    tiles += [(qb, i, i * P) for i in range(KPC)]
                    nlast = len(tiles) - 1
                    h2 = len(tiles) // 2
                    fs_h2 = tiles[h2][2]
                    st = {}
                    for j, (kc, i, fs) in enumerate(tiles):
                        micros.append(
                            (
                                2 * (QB - fs),
                                lambda st=st, h=h, j=j, kc=kc, i=i, fs=fs, nlast=nlast, h2=h2: kb_unit(
                                    st, h, j, kc, i, fs, nlast, h2
                                ),
                            )
                        )
                    micros.append(
                        (QB, lambda st=st, h=h, fs_h2=fs_h2: head_tail(st, h, fs_h2))
                    )
                return micros

            def oproj_micros(co):
                """(cost, fn) micro-units for o_proj of chunk co.

                ha-outer over tsub pairs: each attn tile is consumed by two
                matmuls right after its DMA, so the stream pool stays tiny."""
                t0 = co * QB
                st = {}
                micros = []

                def o_mm(pair, ha0):
                    ta, tb = 2 * pair, 2 * pair + 1
                    if ha0 == 0:
                        st["ya"] = psy_p.tile([P, DQ], F32, tag="yps", name="yps")
                        st["yb"] = psy_p.tile([P, DQ], F32, tag="yps", name="yps2")
                    for ha in range(ha0, ha0 + GP):
                        a = as_p.tile([P, QB], BF16, tag="astr")
                        nc.sync.dma_start(
                            out=a, in_=attn_full[co][ha * P : (ha + 1) * P, :]
                        )
                        g, sub = divmod(ha, GP)
                        nc.tensor.matmul(
                            st["ya"],
                            a[:, ta * P : (ta + 1) * P],
                            wo_sb[g][:, sub * DQ : (sub + 1) * DQ],
                            start=(ha == 0),
                            stop=(ha == HB - 1),
                        )
                        nc.tensor.matmul(
                            st["yb"],
                            a[:, tb * P : (tb + 1) * P],
                            wo_sb[g][:, sub * DQ : (sub + 1) * DQ],
                            start=(ha == 0),
                            stop=(ha == HB - 1),
                        )
                    if ha0 + GP == HB:
                        for tsub, key in ((ta, "ya"), (tb, "yb")):
                            y_sb = y_p.tile([P, DQ], F32, tag="ysb", name=f"ysb{tsub}")
                            if tsub % 2 == 0:
                                nc.scalar.copy(y_sb, st[key])
                            else:
                                nc.vector.tensor_copy(y_sb, st[key])
                            nc.sync.dma_start(
                                out=y_c[t0 + tsub * P : t0 + (tsub + 1) * P, :],
                                in_=y_sb,
                            )

                for pair in range(KPC // 2):
                    for ha0 in range(0, HB, GP):
                        micros.append((2 * GP * DQ, lambda p=pair, a=ha0: o_mm(p, a)))
                return micros

            for it in range(TB + 2):
                c, ca, co = it, it - 1, it - 2
                streams = []
                if 0 <= ca < TB:
                    streams.append((attn_micros(ca), 2.0, 0.0))
                if c < TB:
                    streams.append((proj_micros(c), 1.0, 0.0))
                if it == 0:
                    # deferred so chunk-0 x/cos DMAs land before these
                    for g in range(HG):
                        nc.sync.dma_start(out=wk_sb[g], in_=wk_c[g * P : (g + 1) * P, :])
                        nc.sync.dma_start(out=wv_sb[g], in_=wv_c[g * P : (g + 1) * P, :])
                        nc.sync.dma_start(out=wo_sb[g], in_=wo_c[g * P : (g + 1) * P, :])
                if 0 <= co < TB:
                    streams.append((oproj_micros(co), 1.0, 0.35))
                _weave(streams)
                if 0 <= ca < TB:
                    nc.gpsimd.collective_compute(
                        "AllGather",
                        mybir.AluOpType.bypass,
                        replica_groups=[list(range(NCORES))],
                        ins=[attn_loc[ca][:, :]],
                        outs=[attn_full[ca][:, :]],
                    )
                if ca - 1 >= 0:
                    del qt_tiles[ca - 1]
                    del xt_tiles[ca - 1]
    nc.finalize()
    return nc


def _prep_inputs(hidden_states, wq, wk, wv, wo, position_ids, B, S, H, NH, NKV):
    """Host-side: bf16 casts, x pre-transpose, packed layouts, rope tables."""
    BT = B * S
    NHC = NH // NCORES
    DQ = NHC * P
    HB = H // P
    HG = HB // GP
    TB = BT // QB

    bf = ml_dtypes.bfloat16

    def pack_w(w):
        # [H, D] -> [HG*P, GP*D]: row g*P+p, col sub*D+d = w[(g*GP+sub)*P+p, d]
        D = w.shape[1]
        return np.ascontiguousarray(
            w.reshape(HG, GP, P, D).transpose(0, 2, 1, 3).reshape(HG * P, GP * D)
        )

    xT = np.asarray(hidden_states).reshape(BT, H).T.astype(bf)  # [H, BT]
    # [H, BT] -> [HG*P, TB*GP*QB]: row g*P+p, col c*GP*QB + sub*QB + t
    #   = xT[(g*GP+sub)*P + p, c*QB + t]
    xTp = np.ascontiguousarray(
        xT.reshape(HG, GP, P, TB, QB).transpose(0, 2, 3, 1, 4).reshape(HG * P, TB * GP * QB)
    )
    wq_b, wk_b, wv_b, wo_b = (np.asarray(w).astype(bf) for w in (wq, wk, wv, wo))

    # rope tables in transposed layout [128 d, BT t]
    half = P // 2
    inv_freq = 1.0 / (10000.0 ** (np.arange(half, dtype=np.float64) / half))
    pos = np.asarray(position_ids).astype(np.float64).reshape(BT)  # [b*S+s]
    ang = pos[None, :] * inv_freq[:, None]  # [64, BT]
    cos_t = np.concatenate([np.cos(ang), np.cos(ang)], 0).astype(np.float32)
    sinx_t = np.concatenate([-np.sin(ang), np.sin(ang)], 0).astype(np.float32)

    # tri[k, q] = 1 if k <= q (diagonal 128x128 causal block)
    kk = np.arange(P)[:, None]
    qq = np.arange(P)[None, :]
    tri = (kk <= qq).astype(bf)
    consts_t = np.stack([tri, np.ones((P, P), dtype=bf)]).astype(bf)

    in_maps = []
    for c in range(NCORES):
        in_maps.append(
            {
                "xT_d": xTp,
                "wq_c": pack_w(wq_b[:, c * DQ : (c + 1) * DQ]),
                "wk_c": pack_w(wk_b[:, c * P : (c + 1) * P]),
                "wv_c": pack_w(wv_b[:, c * P : (c + 1) * P]),
                "wo_c": pack_w(wo_b[:, c * DQ : (c + 1) * DQ]),
                "cos_t": cos_t,
                "sinx_t": sinx_t,
                "consts_t": consts_t,
            }
        )
    return in_maps


def run(hidden_states, wq, wk, wv, wo, position_ids, B, S, H, NH, NKV, trace=False):
    key = (B, S, H, NH, NKV)
    if key not in _CACHE:
        _CACHE[key] = build_program(B, S, H, NH, NKV)
    nc = _CACHE[key]
    in_maps = _prep_inputs(
        hidden_states, wq, wk, wv, wo, position_ids, B, S, H, NH, NKV
    )
    res = run_bass_kernel_spmd(nc, in_maps, core_ids=list(range(NCORES)), trace=trace)
    y = np.concatenate([res.results[c]["y_c"] for c in range(NCORES)], axis=1)
    out = y.reshape(B, S, NH * P).astype(np.float32)
    return (out, res) if trace else (out, None)


def kernel(hidden_states, wq, wk, wv, wo, position_ids):
    out, _ = run(
        hidden_states, wq, wk, wv, wo, position_ids, 2, 2048, 4096, 32, 8
    )
    return out


# revision 26
# speedup vs baseline: 1.1966x; 1.0995x over previous
"""Tensor-parallel LlamaAttention (GQA + RoPE + causal) for 8 trn2 NeuronCores.

Sharding: column-parallel q/k/v by head (NH/8 q-heads, NKV/8 kv-heads per
core), attention computed locally per head, chunked AllGather of the
(transposed) attention output across cores overlapped with compute, then
column-parallel o_proj (each core computes a DQ-wide output-column slice);
host concatenates slices.

v3 design:
  - x pre-transposed on the host (xT[h, t]); x and weights packed so every
    DMA line is 4KB (4 h-blocks per SBUF tile) -> descriptor-rate no longer
    gates startup or per-chunk streaming.
  - softmax denominator L via two parallel DVE accumulation chains over the
    P tiles + ONE ones-matmul per (b, head, q-block).
  - causal trim on the diagonal 512-chunk (k-tile i only computes
    q-columns [i*128, 512); only the first 128-col block gets masked).
  - single pool scope, chunk-pipelined, THREE-way cost-weighted weave of
    [attention(c-1), projections(c), o_proj(c-2)] micro-units so the PE
    always has independent matmuls in flight during ACT/DVE latencies;
    gather(c-1) issued at iteration end, fully overlapped.

Layout (token dim = free dim everywhere except v/y):
  qT/kT[d, t] from projection matmuls (lhsT=W block, rhs=xT block) + rope
  v[t, d]     natural layout (lhsT=xT block, rhs=Wv block)
  S^T[k, q]   = matmul(lhsT=kT slice, rhs=qT slice)
  P^T         = exp(scale*S^T) via ACT (scores bounded, exp safe in f32)
  O^T[d, q]  += matmul(lhsT=v tile, rhs=P^T)   (PSUM accumulation over k)
  L[*, q]     = matmul(lhsT=ones, rhs=Pacc)
  attnT       = O^T * reciprocal(L) -> chunked AllGather -> o_proj lhsT
"""

import math
import sys
from contextlib import ExitStack

import numpy as np

sys.path.insert(0, "/opt/trn_rl_repo")

import ml_dtypes  # noqa: E402

from concourse import bacc, mybir, tile  # noqa: E402
from concourse.bass_utils import run_bass_kernel_spmd  # noqa: E402

F32 = mybir.dt.float32
BF16 = mybir.dt.bfloat16
NCORES = 8
P = 128  # partitions / head dim
QB = 512  # q-block / token chunk (PSUM free dim)
GP = 4  # h-blocks packed per DMA tile

_CACHE = {}


def _weave(streams):
    """Merge [(units, weight, offset), ...] preserving intra-list order,
    interleaved by cumulative-cost fraction, and call the fns.

    weight > 1 makes a stream finish earlier (its fraction advances slower
    per unit); offset > 0 holds a stream back until the others reach that
    fraction of their own progress."""
    streams = [s for s in streams if s[0]]
    denoms = [max(1, sum(c for c, _ in s[0])) * wt for s, wt in ((s, s[1]) for s in streams)]
    done = [s[2] * d for s, d in zip(streams, denoms)]
    idx = [0] * len(streams)
    while True:
        best, bf = -1, None
        for si, (s, _, _) in enumerate(streams):
            if idx[si] < len(s):
                f = done[si] / denoms[si]
                if bf is None or f < bf:
                    best, bf = si, f
        if best < 0:
            break
        c, fn = streams[best][0][idx[best]]
        idx[best] += 1
        done[best] += c
        fn()


def build_program(B, S, H, NH, NKV):
    """Build the per-core Bass program. All cores run the same program on
    different weight slices."""
    nc = bacc.Bacc("TRN2", num_devices=NCORES)

    BT = B * S  # total tokens
    NHC = NH // NCORES  # q heads per core
    NKC = NKV // NCORES  # kv heads per core
    assert NKC == 1
    DQ = NHC * P  # per-core q width
    HB = H // P  # h blocks (contraction tiles)
    HG = HB // GP  # packed h groups
    assert HG * GP == HB
    TB = BT // QB  # token chunks
    QBB = S // QB  # q chunks per batch
    KPC = QB // P  # k-tiles (128) per chunk (4)

    xT_d = nc.declare_dram_parameter("xT_d", [HG * P, TB * GP * QB], BF16, isOutput=False)
    wq_c = nc.declare_dram_parameter("wq_c", [HG * P, GP * DQ], BF16, isOutput=False)
    wk_c = nc.declare_dram_parameter("wk_c", [HG * P, GP * P], BF16, isOutput=False)
    wv_c = nc.declare_dram_parameter("wv_c", [HG * P, GP * P], BF16, isOutput=False)
    wo_c = nc.declare_dram_parameter("wo_c", [HG * P, GP * DQ], BF16, isOutput=False)
    cos_t = nc.declare_dram_parameter("cos_t", [P, BT], BF16, isOutput=False)
    sinx_t = nc.declare_dram_parameter("sinx_t", [P, BT], BF16, isOutput=False)
    consts_t = nc.declare_dram_parameter("consts_t", [2, P, P], BF16, isOutput=False)
    y_c = nc.declare_dram_parameter("y_c", [BT, DQ], F32, isOutput=True)

    scale = 1.0 / math.sqrt(P)

    with tile.TileContext(nc) as tc:
        with ExitStack() as stack:
            pool = lambda *a, **kw: stack.enter_context(tc.tile_pool(*a, **kw))
            dram = pool(name="dram", bufs=1, space="DRAM")
            constp = pool(name="const", bufs=1)
            w_p = pool(name="wpool", bufs=1)
            kv_p = pool(name="kvq", bufs=1)
            qt_p = pool(name="qt", bufs=2)
            xt_p = pool(name="xt", bufs=9)
            tab_p = pool(name="tabs", bufs=2)
            rt_p = pool(name="ropetmp", bufs=1)
            p_p = pool(name="pP", bufs=6)
            pa_p = pool(name="pacc", bufs=2)
            pa2_p = pool(name="pacc2", bufs=2)
            ri_p = pool(name="rinvp", bufs=2)
            ao_p = pool(name="aout", bufs=2)
            as_p = pool(name="astr", bufs=33)
            y_p = pool(name="yout", bufs=2)
            psp_p = pool(name="psP", bufs=2, space="PSUM")
            pss_p = pool(name="psS", bufs=2, space="PSUM")
            pso_p = pool(name="psO", bufs=2, space="PSUM")
            psy_p = pool(name="psY", bufs=2, space="PSUM")

            attn_loc = [
                dram.tile([DQ, QB], BF16, tag=f"aloc{c}", name=f"aloc{c}")
                for c in range(TB)
            ]
            attn_full = [
                dram.tile([NCORES * DQ, QB], BF16, tag=f"afull{c}", name=f"afull{c}")
                for c in range(TB)
            ]
            # last chunk gathers per head so the pieces overlap its attention
            attn_loch = [
                dram.tile([P, QB], BF16, tag=f"aloch{h}", name=f"aloch{h}")
                for h in range(NHC)
            ]
            attn_fullh = [
                dram.tile([NCORES * P, QB], BF16, tag=f"afullh{h}", name=f"afullh{h}")
                for h in range(NHC)
            ]

            tri_sb = constp.tile([P, P], BF16, tag="tri")
            nc.sync.dma_start(out=tri_sb, in_=consts_t[0])
            ones_sb = constp.tile([P, P], BF16, tag="ones")
            nc.sync.dma_start(out=ones_sb, in_=consts_t[1])

            # weights resident in SBUF, packed GP h-blocks per tile (4KB lines)
            wq_sb = [w_p.tile([P, GP * DQ], BF16, tag=f"wq{g}", name=f"wq{g}") for g in range(HG)]
            wk_sb = [w_p.tile([P, GP * P], BF16, tag=f"wk{g}", name=f"wk{g}") for g in range(HG)]
            wv_sb = [w_p.tile([P, GP * P], BF16, tag=f"wv{g}", name=f"wv{g}") for g in range(HG)]
            wo_sb = [w_p.tile([P, GP * DQ], BF16, tag=f"wo{g}", name=f"wo{g}") for g in range(HG)]
            for g in range(HG):
                nc.sync.dma_start(out=wq_sb[g], in_=wq_c[g * P : (g + 1) * P, :])

            # persistent per-chunk activations (bf16)
            kT = [kv_p.tile([P, QB], BF16, tag=f"kT{c}", name=f"kT{c}") for c in range(TB)]
            vsb = [kv_p.tile([P, QB], BF16, tag=f"v{c}", name=f"v{c}") for c in range(TB)]
            qt_tiles = {}
            xt_tiles = {}

            def rope(dst, ps, cos_sb, sinx_sb):
                """dst = ps*cos + shift64(ps)*sinx (all [128,QB])"""
                t1 = rt_p.tile([P, QB], F32, tag="ropet1")
                t2 = rt_p.tile([P, QB], F32, tag="ropet2")
                nc.vector.tensor_tensor(t1, ps, cos_sb, mybir.AluOpType.mult)
                hf = P // 2
                nc.vector.tensor_tensor(t2[0:hf], ps[hf:P], sinx_sb[0:hf], mybir.AluOpType.mult)
                nc.vector.tensor_tensor(t2[hf:P], ps[0:hf], sinx_sb[hf:P], mybir.AluOpType.mult)
                nc.vector.tensor_tensor(dst, t1, t2, mybir.AluOpType.add)

            def proj_micros(c):
                """(cost, fn) micro-units for projections of chunk c."""
                t0 = c * GP * QB
                cos_sb = tab_p.tile([P, QB], BF16, tag="cos")
                sinx_sb = tab_p.tile([P, QB], BF16, tag="sinx")
                nc.sync.dma_start(out=cos_sb, in_=cos_t[:, c * QB : (c + 1) * QB])
                nc.sync.dma_start(out=sinx_sb, in_=sinx_t[:, c * QB : (c + 1) * QB])
                xts = []
                for g in range(HG):
                    xi = xt_p.tile([P, GP * QB], BF16, tag="xt")
                    nc.sync.dma_start(
                        out=xi, in_=xT_d[g * P : (g + 1) * P, t0 : t0 + GP * QB]
                    )
                    xts.append(xi)
                xt_tiles[c] = xts

                def xs(hb):  # rhs [P, QB] for h-block hb
                    g, sub = divmod(hb, GP)
                    return xts[g][:, sub * QB : (sub + 1) * QB]

                qts = []
                for h in range(NHC):
                    qts.append(qt_p.tile([P, QB], BF16, tag=f"qT{h}", name=f"qT{h}"))
                qt_tiles[c] = qts

                micros = []
                # q projections, 4-MM micro-units
                state = {}

                def q_mm(h, hb0):
                    if hb0 == 0:
                        state[h] = psp_p.tile([P, QB], F32, tag="pps", name=f"qps{h}")
                    q_ps = state[h]
                    for hb in range(hb0, hb0 + GP):
                        g, sub = divmod(hb, GP)
                        nc.tensor.matmul(
                            q_ps,
                            wq_sb[g][:, sub * DQ + h * P : sub * DQ + (h + 1) * P],
                            xs(hb),
                            start=(hb == 0),
                            stop=(hb == HB - 1),
                        )
                    if hb0 + GP == HB:
                        rope(qts[h], q_ps, cos_sb, sinx_sb)

                for h in range(NHC):
                    for hb0 in range(0, HB, GP):
                        micros.append((GP * QB, lambda h=h, hb0=hb0: q_mm(h, hb0)))

                def k_mm(hb0):
                    if hb0 == 0:
                        state["k"] = psp_p.tile([P, QB], F32, tag="pps", name="kps")
                    k_ps = state["k"]
                    for hb in range(hb0, hb0 + GP):
                        g, sub = divmod(hb, GP)
                        nc.tensor.matmul(
                            k_ps,
                            wk_sb[g][:, sub * P : (sub + 1) * P],
                            xs(hb),
                            start=(hb == 0),
                            stop=(hb == HB - 1),
                        )
                    if hb0 + GP == HB:
                        rope(kT[c], k_ps, cos_sb, sinx_sb)

                for hb0 in range(0, HB, GP):
                    micros.append((GP * QB, lambda hb0=hb0: k_mm(hb0)))

                def v_mm(i, hb0):
                    if i == 0 and hb0 == 0:
                        state["v"] = psp_p.tile([P, QB], F32, tag="pps", name="vps")
                    v_ps = state["v"]
                    for hb in range(hb0, hb0 + GP):
                        g, sub = divmod(hb, GP)
                        nc.tensor.matmul(
                            v_ps[:, i * P : (i + 1) * P],
                            xts[g][:, sub * QB + i * P : sub * QB + (i + 1) * P],
                            wv_sb[g][:, sub * P : (sub + 1) * P],
                            start=(hb == 0),
                            stop=(hb == HB - 1),
                        )
                    if i == KPC - 1 and hb0 + GP == HB:
                        nc.vector.tensor_copy(vsb[c], v_ps)

                for i in range(KPC):
                    for hb0 in range(0, HB, GP):
                        micros.append((GP * P, lambda i=i, hb0=hb0: v_mm(i, hb0)))
                return micros

            def attn_micros(ca):
                """(cost, fn) micro-units for attention of chunk ca."""
                b, qb = divmod(ca, QBB)
                micros = []

                def s_unit(st, h, j, kc, i, fs):
                    if j == 0:
                        st["o"] = pso_p.tile([P, QB], F32, tag="ops", name="ops")
                        st["a0"] = pa_p.tile([P, QB], F32, tag="pacc", name="pacc")
                        st["a1"] = pa2_p.tile([P, QB], F32, tag="pacc2", name="pacc2")
                    kch = b * QBB + kc
                    s_ps = pss_p.tile([P, QB], F32, tag="sps")
                    nc.tensor.matmul(
                        s_ps[:, fs:],
                        kT[kch][:, i * P : (i + 1) * P],
                        qt_tiles[ca][h][:, fs:],
                        start=True,
                        stop=True,
                    )
                    p_sb = p_p.tile([P, QB], BF16, tag="P")
                    nc.scalar.activation(
                        p_sb[:, fs:],
                        s_ps[:, fs:],
                        mybir.ActivationFunctionType.Exp,
                        scale=scale,
                    )
                    st["p"] = p_sb

                def pv_unit(st, j, kc, i, fs, nlast, h2):
                    p_sb = st["p"]
                    if kc == qb:  # diagonal chunk: mask first 128-col block
                        nc.vector.tensor_tensor(
                            p_sb[:, fs : fs + P],
                            p_sb[:, fs : fs + P],
                            tri_sb,
                            mybir.AluOpType.mult,
                        )
                    acc = st["a0"] if j < h2 else st["a1"]
                    if j == 0 or j == h2:
                        nc.vector.tensor_copy(acc[:, fs:], p_sb[:, fs:])
                    else:
                        nc.vector.tensor_tensor(
                            acc[:, fs:], acc[:, fs:], p_sb[:, fs:], mybir.AluOpType.add
                        )
                    kch = b * QBB + kc
                    nc.tensor.matmul(
                        st["o"][:, fs:],
                        vsb[kch][:, i * P : (i + 1) * P],
                        p_sb[:, fs:],
                        start=(j == 0),
                        stop=(j == nlast),
                    )

                def head_tail(st, h, fs_h2):
                    nc.vector.tensor_tensor(
                        st["a0"][:, fs_h2:],
                        st["a0"][:, fs_h2:],
                        st["a1"][:, fs_h2:],
                        mybir.AluOpType.add,
                    )
                    pacc_bf = p_p.tile([P, QB], BF16, tag="P", name="paccb")
                    nc.vector.tensor_copy(pacc_bf, st["a0"])
                    l_ps = pss_p.tile([P, QB], F32, tag="sps")
                    nc.tensor.matmul(l_ps, ones_sb, pacc_bf, start=True, stop=True)
                    rinv = ri_p.tile([P, QB], F32, tag="rinv")
                    nc.vector.reciprocal(rinv, l_ps)
                    attn_sb = ao_p.tile([P, QB], BF16, tag="attn")
                    nc.vector.tensor_tensor(attn_sb, st["o"], rinv, mybir.AluOpType.mult)
                    if ca == TB - 1:
                        nc.sync.dma_start(out=attn_loch[h][:, :], in_=attn_sb)
                        nc.gpsimd.collective_compute(
                            "AllGather",
                            mybir.AluOpType.bypass,
                            replica_groups=[list(range(NCORES))],
                            ins=[attn_loch[h][:, :]],
                            outs=[attn_fullh[h][:, :]],
                        )
                    else:
                        nc.sync.dma_start(
                            out=attn_loc[ca][h * P : (h + 1) * P, :], in_=attn_sb
                        )

                for h in range(NHC):
                    tiles = [(kc, i, 0) for kc in range(qb) for i in range(KPC)]
                    tiles += [(qb, i, i * P) for i in range(KPC)]
                    nlast = len(tiles) - 1
                    h2 = len(tiles) // 2
                    fs_h2 = tiles[h2][2]
                    st = {}
                    for j, (kc, i, fs) in enumerate(tiles):
                        micros.append(
                            (
                                QB - fs,
                                lambda st=st, h=h, j=j, kc=kc, i=i, fs=fs: s_unit(
                                    st, h, j, kc, i, fs
                                ),
                            )
                        )
                        micros.append(
                            (
                                QB - fs,
                                lambda st=st, j=j, kc=kc, i=i, fs=fs, nlast=nlast, h2=h2: pv_unit(
                                    st, j, kc, i, fs, nlast, h2
                                ),
                            )
                        )
                    micros.append(
                        (QB, lambda st=st, h=h, fs_h2=fs_h2: head_tail(st, h, fs_h2))
                    )
                return micros

            def oproj_micros(co):
                """(cost, fn) micro-units for o_proj of chunk co. Attn tiles
                loaded once (tsub==0) and held across all four tsub passes."""
                t0 = co * QB
                af_tiles = [None] * HB
                st = {}
                micros = []

                def af_src(ha):
                    if co == TB - 1:  # last chunk gathered per head
                        cr, h = divmod(ha, NHC)
                        return attn_fullh[h][cr * P : (cr + 1) * P, :]
                    return attn_full[co][ha * P : (ha + 1) * P, :]

                def o_mm(tsub, ha0):
                    if ha0 == 0:
                        st["y"] = psy_p.tile([P, DQ], F32, tag="yps", name="yps")
                    y_ps = st["y"]
                    for ha in range(ha0, ha0 + GP):
                        if tsub == 0:
                            a = as_p.tile([P, QB], BF16, tag="astr")
                            nc.sync.dma_start(out=a, in_=af_src(ha))
                            af_tiles[ha] = a
                        g, sub = divmod(ha, GP)
                        nc.tensor.matmul(
                            y_ps,
                            af_tiles[ha][:, tsub * P : (tsub + 1) * P],
                            wo_sb[g][:, sub * DQ : (sub + 1) * DQ],
                            start=(ha == 0),
                            stop=(ha == HB - 1),
                        )
                    if ha0 + GP == HB:
                        y_sb = y_p.tile([P, DQ], F32, tag="ysb", name="ysb")
                        if tsub % 2 == 0:
                            nc.scalar.copy(y_sb, y_ps)
                        else:
                            nc.vector.tensor_copy(y_sb, y_ps)
                        nc.sync.dma_start(
                            out=y_c[t0 + tsub * P : t0 + (tsub + 1) * P, :], in_=y_sb
                        )

                for tsub in range(KPC):
                    for ha0 in range(0, HB, GP):
                        micros.append((GP * DQ, lambda t=tsub, a=ha0: o_mm(t, a)))
                return micros

            for it in range(TB + 2):
                c, ca, co = it, it - 1, it - 2
                streams = []
                if 0 <= ca < TB:
                    streams.append((attn_micros(ca), 2.0, 0.0))
                if c < TB:
                    streams.append((proj_micros(c), 1.0, 0.0))
                if it == 0:
                    # deferred so chunk-0 x/cos DMAs land before these
                    for g in range(HG):
                        nc.sync.dma_start(out=wk_sb[g], in_=wk_c[g * P : (g + 1) * P, :])
                        nc.sync.dma_start(out=wv_sb[g], in_=wv_c[g * P : (g + 1) * P, :])
                        nc.sync.dma_start(out=wo_sb[g], in_=wo_c[g * P : (g + 1) * P, :])
                if 0 <= co < TB:
                    streams.append((oproj_micros(co), 1.0, 0.45))
                _weave(streams)
                if 0 <= ca < TB - 1:
                    nc.gpsimd.collective_compute(
                        "AllGather",
                        mybir.AluOpType.bypass,
                        replica_groups=[list(range(NCORES))],
                        ins=[attn_loc[ca][:, :]],
                        outs=[attn_full[ca][:, :]],
                    )
                if ca - 1 >= 0:
                    del qt_tiles[ca - 1]
                    del xt_tiles[ca - 1]
    nc.finalize()
    return nc


def _prep_inputs(hidden_states, wq, wk, wv, wo, position_ids, B, S, H, NH, NKV):
    """Host-side: bf16 casts, x pre-transpose, packed layouts, rope tables."""
    BT = B * S
    NHC = NH // NCORES
    DQ = NHC * P
    HB = H // P
    HG = HB // GP
    TB = BT // QB

    bf = ml_dtypes.bfloat16

    def pack_w(w):
        # [H, D] -> [HG*P, GP*D]: row g*P+p, col sub*D+d = w[(g*GP+sub)*P+p, d]
        D = w.shape[1]
        return np.ascontiguousarray(
            w.reshape(HG, GP, P, D).transpose(0, 2, 1, 3).reshape(HG * P, GP * D)
        )

    xT = np.asarray(hidden_states).reshape(BT, H).T.astype(bf)  # [H, BT]
    # [H, BT] -> [HG*P, TB*GP*QB]: row g*P+p, col c*GP*QB + sub*QB + t
    #   = xT[(g*GP+sub)*P + p, c*QB + t]
    xTp = np.ascontiguousarray(
        xT.reshape(HG, GP, P, TB, QB).transpose(0, 2, 3, 1, 4).reshape(HG * P, TB * GP * QB)
    )
    wq_b, wk_b, wv_b, wo_b = (np.asarray(w).astype(bf) for w in (wq, wk, wv, wo))

    # rope tables in transposed layout [128 d, BT t]
    half = P // 2
    inv_freq = 1.0 / (10000.0 ** (np.arange(half, dtype=np.float64) / half))
    pos = np.asarray(position_ids).astype(np.float64).reshape(BT)  # [b*S+s]
    ang = pos[None, :] * inv_freq[:, None]  # [64, BT]
    cos_t = np.concatenate([np.cos(ang), np.cos(ang)], 0).astype(bf)
    sinx_t = np.concatenate([-np.sin(ang), np.sin(ang)], 0).astype(bf)

    # tri[k, q] = 1 if k <= q (diagonal 128x128 causal block)
    kk = np.arange(P)[:, None]
    qq = np.arange(P)[None, :]
    tri = (kk <= qq).astype(bf)
    consts_t = np.stack([tri, np.ones((P, P), dtype=bf)]).astype(bf)

    in_maps = []
    for c in range(NCORES):
        in_maps.append(
            {
                "xT_d": xTp,
                "wq_c": pack_w(wq_b[:, c * DQ : (c + 1) * DQ]),
                "wk_c": pack_w(wk_b[:, c * P : (c + 1) * P]),
                "wv_c": pack_w(wv_b[:, c * P : (c + 1) * P]),
                "wo_c": pack_w(wo_b[:, c * DQ : (c + 1) * DQ]),
                "cos_t": cos_t,
                "sinx_t": sinx_t,
                "consts_t": consts_t,
            }
        )
    return in_maps


def run(hidden_states, wq, wk, wv, wo, position_ids, B, S, H, NH, NKV, trace=False):
    key = (B, S, H, NH, NKV)
    if key not in _CACHE:
        _CACHE[key] = build_program(B, S, H, NH, NKV)
    nc = _CACHE[key]
    in_maps = _prep_inputs(
        hidden_states, wq, wk, wv, wo, position_ids, B, S, H, NH, NKV
    )
    res = run_bass_kernel_spmd(nc, in_maps, core_ids=list(range(NCORES)), trace=trace)
    y = np.concatenate([res.results[c]["y_c"] for c in range(NCORES)], axis=1)
    out = y.reshape(B, S, NH * P).astype(np.float32)
    return (out, res) if trace else (out, None)


def kernel(hidden_states, wq, wk, wv, wo, position_ids):
    out, _ = run(
        hidden_states, wq, wk, wv, wo, position_ids, 2, 2048, 4096, 32, 8
    )
    return out


# revision 28
# speedup vs baseline: 1.2236x; 1.0226x over previous
"""Tensor-parallel LlamaAttention (GQA + RoPE + causal) for 8 trn2 NeuronCores.

Sharding: column-parallel q/k/v by head (NH/8 q-heads, NKV/8 kv-heads per
core), attention computed locally per head, chunked AllGather of the
(transposed) attention output across cores overlapped with compute, then
column-parallel o_proj (each core computes a DQ-wide output-column slice);
host concatenates slices.

v3 design:
  - x pre-transposed on the host (xT[h, t]); x and weights packed so every
    DMA line is 4KB (4 h-blocks per SBUF tile) -> descriptor-rate no longer
    gates startup or per-chunk streaming.
  - softmax denominator L via two parallel DVE accumulation chains over the
    P tiles + ONE ones-matmul per (b, head, q-block).
  - causal trim on the diagonal 512-chunk (k-tile i only computes
    q-columns [i*128, 512); only the first 128-col block gets masked).
  - single pool scope, chunk-pipelined, THREE-way cost-weighted weave of
    [attention(c-1), projections(c), o_proj(c-2)] micro-units so the PE
    always has independent matmuls in flight during ACT/DVE latencies;
    gather(c-1) issued at iteration end, fully overlapped.

Layout (token dim = free dim everywhere except v/y):
  qT/kT[d, t] from projection matmuls (lhsT=W block, rhs=xT block) + rope
  v[t, d]     natural layout (lhsT=xT block, rhs=Wv block)
  S^T[k, q]   = matmul(lhsT=kT slice, rhs=qT slice)
  P^T         = exp(scale*S^T) via ACT (scores bounded, exp safe in f32)
  O^T[d, q]  += matmul(lhsT=v tile, rhs=P^T)   (PSUM accumulation over k)
  L[*, q]     = matmul(lhsT=ones, rhs=Pacc)
  attnT       = O^T * reciprocal(L) -> chunked AllGather -> o_proj lhsT
"""

import math
import sys
from contextlib import ExitStack

import numpy as np

sys.path.insert(0, "/opt/trn_rl_repo")

import ml_dtypes  # noqa: E402

from concourse import bacc, mybir, tile  # noqa: E402
from concourse.bass_utils import run_bass_kernel_spmd  # noqa: E402

F32 = mybir.dt.float32
BF16 = mybir.dt.bfloat16
NCORES = 8
P = 128  # partitions / head dim
QB = 512  # q-block / token chunk (PSUM free dim)
GP = 4  # h-blocks packed per DMA tile

_CACHE = {}


def _weave(streams):
    """Merge [(units, weight, offset), ...] preserving intra-list order,
    interleaved by cumulative-cost fraction, and call the fns.

    weight > 1 makes a stream finish earlier (its fraction advances slower
    per unit); offset > 0 holds a stream back until the others reach that
    fraction of their own progress."""
    streams = [s for s in streams if s[0]]
    denoms = [max(1, sum(c for c, _ in s[0])) * wt for s, wt in ((s, s[1]) for s in streams)]
    done = [s[2] * d for s, d in zip(streams, denoms)]
    idx = [0] * len(streams)
    while True:
        best, bf = -1, None
        for si, (s, _, _) in enumerate(streams):
            if idx[si] < len(s):
                f = done[si] / denoms[si]
                if bf is None or f < bf:
                    best, bf = si, f
        if best < 0:
            break
        c, fn = streams[best][0][idx[best]]
        idx[best] += 1
        done[best] += c
        fn()


def build_program(B, S, H, NH, NKV):
    """Build the per-core Bass program. All cores run the same program on
    different weight slices."""
    nc = bacc.Bacc("TRN2", num_devices=NCORES)

    BT = B * S  # total tokens
    NHC = NH // NCORES  # q heads per core
    NKC = NKV // NCORES  # kv heads per core
    assert NKC == 1
    DQ = NHC * P  # per-core q width
    HB = H // P  # h blocks (contraction tiles)
    HG = HB // GP  # packed h groups
    assert HG * GP == HB
    TB = BT // QB  # token chunks
    QBB = S // QB  # q chunks per batch
    KPC = QB // P  # k-tiles (128) per chunk (4)

    xT_d = nc.declare_dram_parameter("xT_d", [HG * P, TB * GP * QB], BF16, isOutput=False)
    wq_c = nc.declare_dram_parameter("wq_c", [HG * P, GP * DQ], BF16, isOutput=False)
    wk_c = nc.declare_dram_parameter("wk_c", [HG * P, GP * P], BF16, isOutput=False)
    wv_c = nc.declare_dram_parameter("wv_c", [HG * P, GP * P], BF16, isOutput=False)
    wo_c = nc.declare_dram_parameter("wo_c", [HG * P, GP * DQ], BF16, isOutput=False)
    cos_t = nc.declare_dram_parameter("cos_t", [P, BT], BF16, isOutput=False)
    sinx_t = nc.declare_dram_parameter("sinx_t", [P, BT], BF16, isOutput=False)
    consts_t = nc.declare_dram_parameter("consts_t", [2, P, P], BF16, isOutput=False)
    y_c = nc.declare_dram_parameter("y_c", [BT, DQ], F32, isOutput=True)

    scale = 1.0 / math.sqrt(P)

    with tile.TileContext(nc) as tc:
        with ExitStack() as stack:
            pool = lambda *a, **kw: stack.enter_context(tc.tile_pool(*a, **kw))
            dram = pool(name="dram", bufs=1, space="DRAM")
            constp = pool(name="const", bufs=1)
            w_p = pool(name="wpool", bufs=1)
            kv_p = pool(name="kvq", bufs=1)
            qt_p = pool(name="qt", bufs=2)
            xt_p = pool(name="xt", bufs=9)
            tab_p = pool(name="tabs", bufs=2)
            rt_p = pool(name="ropetmp", bufs=1)
            p_p = pool(name="pP", bufs=6)
            pa_p = pool(name="pacc", bufs=2)
            pa2_p = pool(name="pacc2", bufs=2)
            ri_p = pool(name="rinvp", bufs=2)
            ao_p = pool(name="aout", bufs=2)
            as_p = pool(name="astr", bufs=33)
            y_p = pool(name="yout", bufs=2)
            psp_p = pool(name="psP", bufs=2, space="PSUM")
            pss_p = pool(name="psS", bufs=2, space="PSUM")
            pso_p = pool(name="psO", bufs=2, space="PSUM")
            psy_p = pool(name="psY", bufs=2, space="PSUM")

            attn_loc = [
                dram.tile([DQ, QB], BF16, tag=f"aloc{c}", name=f"aloc{c}")
                for c in range(TB)
            ]
            attn_full = [
                dram.tile([NCORES * DQ, QB], BF16, tag=f"afull{c}", name=f"afull{c}")
                for c in range(TB)
            ]
            # last chunk gathers per head so the pieces overlap its attention
            attn_loch = [
                dram.tile([P, QB], BF16, tag=f"aloch{h}", name=f"aloch{h}")
                for h in range(NHC)
            ]
            attn_fullh = [
                dram.tile([NCORES * P, QB], BF16, tag=f"afullh{h}", name=f"afullh{h}")
                for h in range(NHC)
            ]

            tri_sb = constp.tile([P, P], BF16, tag="tri")
            nc.sync.dma_start(out=tri_sb, in_=consts_t[0])
            ones_sb = constp.tile([P, P], BF16, tag="ones")
            nc.sync.dma_start(out=ones_sb, in_=consts_t[1])

            # weights resident in SBUF, packed GP h-blocks per tile (4KB lines)
            wq_sb = [w_p.tile([P, GP * DQ], BF16, tag=f"wq{g}", name=f"wq{g}") for g in range(HG)]
            wk_sb = [w_p.tile([P, GP * P], BF16, tag=f"wk{g}", name=f"wk{g}") for g in range(HG)]
            wv_sb = [w_p.tile([P, GP * P], BF16, tag=f"wv{g}", name=f"wv{g}") for g in range(HG)]
            wo_sb = [w_p.tile([P, GP * DQ], BF16, tag=f"wo{g}", name=f"wo{g}") for g in range(HG)]
            for g in range(HG):
                nc.sync.dma_start(out=wq_sb[g], in_=wq_c[g * P : (g + 1) * P, :])

            # persistent per-chunk activations (bf16)
            kT = [kv_p.tile([P, QB], BF16, tag=f"kT{c}", name=f"kT{c}") for c in range(TB)]
            vsb = [kv_p.tile([P, QB], BF16, tag=f"v{c}", name=f"v{c}") for c in range(TB)]
            qt_tiles = {}
            xt_tiles = {}

            def rope(dst, ps, cos_sb, sinx_sb):
                """dst = ps*cos + shift64(ps)*sinx (all [128,QB])"""
                t1 = rt_p.tile([P, QB], F32, tag="ropet1")
                t2 = rt_p.tile([P, QB], F32, tag="ropet2")
                nc.vector.tensor_tensor(t1, ps, cos_sb, mybir.AluOpType.mult)
                hf = P // 2
                nc.vector.tensor_tensor(t2[0:hf], ps[hf:P], sinx_sb[0:hf], mybir.AluOpType.mult)
                nc.vector.tensor_tensor(t2[hf:P], ps[0:hf], sinx_sb[hf:P], mybir.AluOpType.mult)
                nc.vector.tensor_tensor(dst, t1, t2, mybir.AluOpType.add)

            def proj_micros(c):
                """(cost, fn) micro-units for projections of chunk c."""
                t0 = c * GP * QB
                cos_sb = tab_p.tile([P, QB], BF16, tag="cos")
                sinx_sb = tab_p.tile([P, QB], BF16, tag="sinx")
                nc.sync.dma_start(out=cos_sb, in_=cos_t[:, c * QB : (c + 1) * QB])
                nc.sync.dma_start(out=sinx_sb, in_=sinx_t[:, c * QB : (c + 1) * QB])
                xts = []
                for g in range(HG):
                    xi = xt_p.tile([P, GP * QB], BF16, tag="xt")
                    nc.sync.dma_start(
                        out=xi, in_=xT_d[g * P : (g + 1) * P, t0 : t0 + GP * QB]
                    )
                    xts.append(xi)
                xt_tiles[c] = xts

                def xs(hb):  # rhs [P, QB] for h-block hb
                    g, sub = divmod(hb, GP)
                    return xts[g][:, sub * QB : (sub + 1) * QB]

                qts = []
                for h in range(NHC):
                    qts.append(qt_p.tile([P, QB], BF16, tag=f"qT{h}", name=f"qT{h}"))
                qt_tiles[c] = qts

                micros = []
                # q projections, 4-MM micro-units
                state = {}

                def q_mm(h, hb0):
                    if hb0 == 0:
                        state[h] = psp_p.tile([P, QB], F32, tag="pps", name=f"qps{h}")
                    q_ps = state[h]
                    for hb in range(hb0, hb0 + GP):
                        g, sub = divmod(hb, GP)
                        nc.tensor.matmul(
                            q_ps,
                            wq_sb[g][:, sub * DQ + h * P : sub * DQ + (h + 1) * P],
                            xs(hb),
                            start=(hb == 0),
                            stop=(hb == HB - 1),
                        )
                    if hb0 + GP == HB:
                        rope(qts[h], q_ps, cos_sb, sinx_sb)

                for h in range(NHC):
                    for hb0 in range(0, HB, GP):
                        micros.append((GP * QB, lambda h=h, hb0=hb0: q_mm(h, hb0)))

                def k_mm(hb0):
                    if hb0 == 0:
                        state["k"] = psp_p.tile([P, QB], F32, tag="pps", name="kps")
                    k_ps = state["k"]
                    for hb in range(hb0, hb0 + GP):
                        g, sub = divmod(hb, GP)
                        nc.tensor.matmul(
                            k_ps,
                            wk_sb[g][:, sub * P : (sub + 1) * P],
                            xs(hb),
                            start=(hb == 0),
                            stop=(hb == HB - 1),
                        )
                    if hb0 + GP == HB:
                        rope(kT[c], k_ps, cos_sb, sinx_sb)

                for hb0 in range(0, HB, GP):
                    micros.append((GP * QB, lambda hb0=hb0: k_mm(hb0)))

                def v_mm(i, hb0):
                    if i == 0 and hb0 == 0:
                        state["v"] = psp_p.tile([P, QB], F32, tag="pps", name="vps")
                    v_ps = state["v"]
                    for hb in range(hb0, hb0 + GP):
                        g, sub = divmod(hb, GP)
                        nc.tensor.matmul(
                            v_ps[:, i * P : (i + 1) * P],
                            xts[g][:, sub * QB + i * P : sub * QB + (i + 1) * P],
                            wv_sb[g][:, sub * P : (sub + 1) * P],
                            start=(hb == 0),
                            stop=(hb == HB - 1),
                        )
                    if i == KPC - 1 and hb0 + GP == HB:
                        nc.vector.tensor_copy(vsb[c], v_ps)

                for i in range(KPC):
                    for hb0 in range(0, HB, GP):
                        micros.append((GP * P, lambda i=i, hb0=hb0: v_mm(i, hb0)))
                return micros

            def attn_micros(ca):
                """(cost, fn) micro-units for attention of chunk ca."""
                b, qb = divmod(ca, QBB)
                micros = []

                def s_unit(st, h, j, kc, i, fs):
                    if j == 0:
                        st["o"] = pso_p.tile([P, QB], F32, tag="ops", name="ops")
                        st["a0"] = pa_p.tile([P, QB], F32, tag="pacc", name="pacc")
                        st["a1"] = pa2_p.tile([P, QB], F32, tag="pacc2", name="pacc2")
                    kch = b * QBB + kc
                    s_ps = pss_p.tile([P, QB], F32, tag="sps")
                    nc.tensor.matmul(
                        s_ps[:, fs:],
                        kT[kch][:, i * P : (i + 1) * P],
                        qt_tiles[ca][h][:, fs:],
                        start=True,
                        stop=True,
                    )
                    p_sb = p_p.tile([P, QB], BF16, tag="P")
                    nc.scalar.activation(
                        p_sb[:, fs:],
                        s_ps[:, fs:],
                        mybir.ActivationFunctionType.Exp,
                        scale=scale,
                    )
                    st["p"] = p_sb

                def pv_unit(st, j, kc, i, fs, nlast, h2):
                    p_sb = st["p"]
                    if kc == qb:  # diagonal chunk: mask first 128-col block
                        nc.vector.tensor_tensor(
                            p_sb[:, fs : fs + P],
                            p_sb[:, fs : fs + P],
                            tri_sb,
                            mybir.AluOpType.mult,
                        )
                    acc = st["a0"] if j < h2 else st["a1"]
                    if j == 0 or j == h2:
                        nc.vector.tensor_copy(acc[:, fs:], p_sb[:, fs:])
                    else:
                        nc.vector.tensor_tensor(
                            acc[:, fs:], acc[:, fs:], p_sb[:, fs:], mybir.AluOpType.add
                        )
                    kch = b * QBB + kc
                    nc.tensor.matmul(
                        st["o"][:, fs:],
                        vsb[kch][:, i * P : (i + 1) * P],
                        p_sb[:, fs:],
                        start=(j == 0),
                        stop=(j == nlast),
                    )

                def head_tail(st, h, fs_h2):
                    nc.vector.tensor_tensor(
                        st["a0"][:, fs_h2:],
                        st["a0"][:, fs_h2:],
                        st["a1"][:, fs_h2:],
                        mybir.AluOpType.add,
                    )
                    pacc_bf = p_p.tile([P, QB], BF16, tag="P", name="paccb")
                    nc.vector.tensor_copy(pacc_bf, st["a0"])
                    l_ps = pss_p.tile([P, QB], F32, tag="sps")
                    nc.tensor.matmul(l_ps, ones_sb, pacc_bf, start=True, stop=True)
                    rinv = ri_p.tile([P, QB], F32, tag="rinv")
                    nc.vector.reciprocal(rinv, l_ps)
                    attn_sb = ao_p.tile([P, QB], BF16, tag="attn")
                    nc.vector.tensor_tensor(attn_sb, st["o"], rinv, mybir.AluOpType.mult)
                    if ca == TB - 1:
                        nc.sync.dma_start(out=attn_loch[h][:, :], in_=attn_sb)
                        nc.gpsimd.collective_compute(
                            "AllGather",
                            mybir.AluOpType.bypass,
                            replica_groups=[list(range(NCORES))],
                            ins=[attn_loch[h][:, :]],
                            outs=[attn_fullh[h][:, :]],
                        )
                    else:
                        nc.sync.dma_start(
                            out=attn_loc[ca][h * P : (h + 1) * P, :], in_=attn_sb
                        )

                pending_tail = None
                for h in range(NHC):
                    tiles = [(kc, i, 0) for kc in range(qb) for i in range(KPC)]
                    tiles += [(qb, i, i * P) for i in range(KPC)]
                    nlast = len(tiles) - 1
                    h2 = len(tiles) // 2
                    fs_h2 = tiles[h2][2]
                    st = {}
                    for j, (kc, i, fs) in enumerate(tiles):
                        micros.append(
                            (
                                QB - fs,
                                lambda st=st, h=h, j=j, kc=kc, i=i, fs=fs: s_unit(
                                    st, h, j, kc, i, fs
                                ),
                            )
                        )
                        micros.append(
                            (
                                QB - fs,
                                lambda st=st, j=j, kc=kc, i=i, fs=fs, nlast=nlast, h2=h2: pv_unit(
                                    st, j, kc, i, fs, nlast, h2
                                ),
                            )
                        )
                        # place the previous head's tail a few units into this
                        # head, giving its DVE chain time to drain first
                        if pending_tail is not None and j == 1:
                            micros.append(pending_tail)
                            pending_tail = None
                    pending_tail = (
                        QB,
                        lambda st=st, h=h, fs_h2=fs_h2: head_tail(st, h, fs_h2),
                    )
                micros.append(pending_tail)
                return micros

            def oproj_micros(co):
                """(cost, fn) micro-units for o_proj of chunk co. Attn tiles
                loaded once (tsub==0) and held across all four tsub passes."""
                t0 = co * QB
                af_tiles = [None] * HB
                st = {}
                micros = []

                def af_src(ha):
                    if co == TB - 1:  # last chunk gathered per head
                        cr, h = divmod(ha, NHC)
                        return attn_fullh[h][cr * P : (cr + 1) * P, :]
                    return attn_full[co][ha * P : (ha + 1) * P, :]

                if co == TB - 1:
                    # contract in head-major order: the first blocks only need
                    # head 0's gather, which lands first
                    order = [cr * NHC + h for h in range(NHC) for cr in range(NCORES)]
                else:
                    order = list(range(HB))

                def o_mm(tsub, i0):
                    if i0 == 0:
                        st["y"] = psy_p.tile([P, DQ], F32, tag="yps", name="yps")
                    y_ps = st["y"]
                    for ii in range(i0, i0 + GP):
                        ha = order[ii]
                        if tsub == 0:
                            a = as_p.tile([P, QB], BF16, tag="astr")
                            nc.sync.dma_start(out=a, in_=af_src(ha))
                            af_tiles[ha] = a
                        g, sub = divmod(ha, GP)
                        nc.tensor.matmul(
                            y_ps,
                            af_tiles[ha][:, tsub * P : (tsub + 1) * P],
                            wo_sb[g][:, sub * DQ : (sub + 1) * DQ],
                            start=(ii == 0),
                            stop=(ii == HB - 1),
                        )
                    if i0 + GP == HB:
                        y_sb = y_p.tile([P, DQ], F32, tag="ysb", name="ysb")
                        if tsub % 2 == 0:
                            nc.scalar.copy(y_sb, y_ps)
                        else:
                            nc.vector.tensor_copy(y_sb, y_ps)
                        nc.sync.dma_start(
                            out=y_c[t0 + tsub * P : t0 + (tsub + 1) * P, :], in_=y_sb
                        )

                for tsub in range(KPC):
                    for i0 in range(0, HB, GP):
                        micros.append((GP * DQ, lambda t=tsub, a=i0: o_mm(t, a)))
                return micros

            for it in range(TB + 2):
                c, ca, co = it, it - 1, it - 2
                streams = []
                if 0 <= ca < TB:
                    streams.append((attn_micros(ca), 2.0, 0.0))
                if c < TB:
                    streams.append((proj_micros(c), 1.0, 0.0))
                if it == 0:
                    # deferred so chunk-0 x/cos DMAs land before these
                    for g in range(HG):
                        nc.sync.dma_start(out=wk_sb[g], in_=wk_c[g * P : (g + 1) * P, :])
                        nc.sync.dma_start(out=wv_sb[g], in_=wv_c[g * P : (g + 1) * P, :])
                        nc.sync.dma_start(out=wo_sb[g], in_=wo_c[g * P : (g + 1) * P, :])
                if 0 <= co < TB:
                    streams.append((oproj_micros(co), 1.0, 0.45))
                _weave(streams)
                if 0 <= ca < TB - 1:
                    nc.gpsimd.collective_compute(
                        "AllGather",
                        mybir.AluOpType.bypass,
                        replica_groups=[list(range(NCORES))],
                        ins=[attn_loc[ca][:, :]],
                        outs=[attn_full[ca][:, :]],
                    )
                if ca - 1 >= 0:
                    del qt_tiles[ca - 1]
                    del xt_tiles[ca - 1]
    nc.finalize()
    return nc


def _prep_inputs(hidden_states, wq, wk, wv, wo, position_ids, B, S, H, NH, NKV):
    """Host-side: bf16 casts, x pre-transpose, packed layouts, rope tables."""
    BT = B * S
    NHC = NH // NCORES
    DQ = NHC * P
    HB = H // P
    HG = HB // GP
    TB = BT // QB

    bf = ml_dtypes.bfloat16

    def pack_w(w):
        # [H, D] -> [HG*P, GP*D]: row g*P+p, col sub*D+d = w[(g*GP+sub)*P+p, d]
        D = w.shape[1]
        return np.ascontiguousarray(
            w.reshape(HG, GP, P, D).transpose(0, 2, 1, 3).reshape(HG * P, GP * D)
        )

    xT = np.asarray(hidden_states).reshape(BT, H).T.astype(bf)  # [H, BT]
    # [H, BT] -> [HG*P, TB*GP*QB]: row g*P+p, col c*GP*QB + sub*QB + t
    #   = xT[(g*GP+sub)*P + p, c*QB + t]
    xTp = np.ascontiguousarray(
        xT.reshape(HG, GP, P, TB, QB).transpose(0, 2, 3, 1, 4).reshape(HG * P, TB * GP * QB)
    )
    wq_b, wk_b, wv_b, wo_b = (np.asarray(w).astype(bf) for w in (wq, wk, wv, wo))

    # rope tables in transposed layout [128 d, BT t]
    half = P // 2
    inv_freq = 1.0 / (10000.0 ** (np.arange(half, dtype=np.float64) / half))
    pos = np.asarray(position_ids).astype(np.float64).reshape(BT)  # [b*S+s]
    ang = pos[None, :] * inv_freq[:, None]  # [64, BT]
    cos_t = np.concatenate([np.cos(ang), np.cos(ang)], 0).astype(bf)
    sinx_t = np.concatenate([-np.sin(ang), np.sin(ang)], 0).astype(bf)

    # tri[k, q] = 1 if k <= q (diagonal 128x128 causal block)
    kk = np.arange(P)[:, None]
    qq = np.arange(P)[None, :]
    tri = (kk <= qq).astype(bf)
    consts_t = np.stack([tri, np.ones((P, P), dtype=bf)]).astype(bf)

    in_maps = []
    for c in range(NCORES):
        in_maps.append(
            {
                "xT_d": xTp,
                "wq_c": pack_w(wq_b[:, c * DQ : (c + 1) * DQ]),
                "wk_c": pack_w(wk_b[:, c * P : (c + 1) * P]),
                "wv_c": pack_w(wv_b[:, c * P : (c + 1) * P]),
                "wo_c": pack_w(wo_b[:, c * DQ : (c + 1) * DQ]),
                "cos_t": cos_t,
                "sinx_t": sinx_t,
                "consts_t": consts_t,
            }
        )
    return in_maps


def run(hidden_states, wq, wk, wv, wo, position_ids, B, S, H, NH, NKV, trace=False):
    key = (B, S, H, NH, NKV)
    if key not in _CACHE:
        _CACHE[key] = build_program(B, S, H, NH, NKV)
    nc = _CACHE[key]
    in_maps = _prep_inputs(
        hidden_states, wq, wk, wv, wo, position_ids, B, S, H, NH, NKV
    )
    res = run_bass_kernel_spmd(nc, in_maps, core_ids=list(range(NCORES)), trace=trace)
    y = np.concatenate([res.results[c]["y_c"] for c in range(NCORES)], axis=1)
    out = y.reshape(B, S, NH * P).astype(np.float32)
    return (out, res) if trace else (out, None)


def kernel(hidden_states, wq, wk, wv, wo, position_ids):
    out, _ = run(
        hidden_states, wq, wk, wv, wo, position_ids, 2, 2048, 4096, 32, 8
    )
    return out
